# revision 11
# baseline (speedup 1.0000x reference)
"""MixHopNetwork Trainium2 kernel: 8-core SPMD Bass/Tile implementation.

Reference computation (N=10000 nodes, F=500, H=400, C=1200):
  h_i = relu(X @ Wu_i); a1 = [h1, adj@h2, adj@adj@h3]            (N, 1200)
  g_i = a1 @ Wb_i;      a2 = [g1, adj@g2, adj@adj@g3]            (N, 1200)
  out = relu(a2 @ W_fc + b_fc)                                   (N, 1200)

Distribution: nodes row-sharded over 8 cores (1280 rows each, padded
10000->10240).  adj is pre-transposed on the host so each core holds the
column-shard adjT[:, mine] with the contraction dim partition-major.
The six adj@h products collapse into 4 adjacency passes (A: adj@[h2|h3],
B: adj@t3, C: adj@[g2|g3], D: adj@v3) with AllGathers of each core's
activation shard between dependent passes.  All matmuls run in bf16 with
fp32 PSUM accumulation.

Overlap structure: the contraction (source-node) order is permuted to
[half][rank][640] so every boundary AllGather splits into two half-shard
gathers -- the first fires mid-pass and overlaps the producing pass's
second half; the consuming pass walks half-0 k-blocks first so the
second gather overlaps its first half.  adjT is stored pre-tiled per
(m-group, k-block) so every stationary-slab DMA is one contiguous read.
rhs operands that are reused across m-groups (h2/g2 halves, t3/v3) stay
SBUF-resident; only the h3/g3 halves are re-streamed per m-group.

Feature-contraction matmuls need feature-major activations; adjacency
passes produce node-major ones.  a1/a2 are assembled feature-major from
directly-transposed matmuls (h1^T, g1^T via out = W^T @ X^T) plus PE
transposes of t2/u3/v2/w3.  Each 400-row concat chunk is padded to 512
rows to keep tiles 128-aligned; matching weight rows are zero-padded on
the host, which also folds b_fc into W_fc as contraction row 1536
against a constant ones-row appended to a2T.
"""
import sys

for _p in ("/opt/trn_rl_repo", "/root/.axon_site/_ro/trn_rl_repo"):
    if _p not in sys.path:
        sys.path.insert(0, _p)

import numpy as np
import ml_dtypes

NCORES = 8
N = 10000            # nodes
NP = 10240           # padded nodes
S = NP // NCORES     # 1280 rows per core
MT = S // 128        # 10 node tiles per core
KT = NP // 128       # 80 contraction tiles
KB = 5               # k-tiles per k-block (640 rows)
NB = KT // KB        # 16 k-blocks = [2 halves][8 ranks]
F, FP = 500, 512
H = 400
C = 1200
CH = 512             # padded chunk height in a1T/a2T
CKT = 3 * CH // 128  # 12 k-tiles over packed feature dim

GA = [3, 2, 3, 2]    # m-tile groups for 800-wide passes (A, C)
GB = [5, 5]          # m-tile groups for 400-wide passes (B, D)

BF16 = ml_dtypes.bfloat16

_compiled = None


def _build():
    from concourse import bass, bacc, tile, mybir
    from concourse import masks

    f32 = mybir.dt.float32
    bf16 = mybir.dt.bfloat16
    RELU = mybir.ActivationFunctionType.Relu

    nc = bacc.Bacc("TRN2", target_bir_lowering=False, debug=False,
                   num_devices=NCORES)

    adjA_d = [nc.dram_tensor(f"adjA{g}", [NB, 128, KB * G * 128], bf16,
                             kind="ExternalInput") for g, G in enumerate(GA)]
    adjB_d = [nc.dram_tensor(f"adjB{g}", [NB, 128, KB * G * 128], bf16,
                             kind="ExternalInput") for g, G in enumerate(GB)]
    xT_d = nc.dram_tensor("xT", [FP, S], bf16, kind="ExternalInput")
    wu_d = nc.dram_tensor("wu", [3, FP, H], bf16, kind="ExternalInput")
    wb_d = nc.dram_tensor("wb", [3, 3 * CH, H], bf16, kind="ExternalInput")
    wfc_d = nc.dram_tensor("wfc", [3 * CH + 128, C], bf16,
                           kind="ExternalInput")
    out_d = nc.dram_tensor("out", [S, C], f32, kind="ExternalOutput")

    RG = [list(range(NCORES))]
    HC = [(0, 128), (128, 128), (256, 128), (384, 16)]   # H chunks
    NCH = [(0, 512), (512, 512), (1024, 256)]            # node chunks

    with tile.TileContext(nc) as tc:
        with (
            tc.tile_pool(name="const", bufs=1) as constp,
            tc.tile_pool(name="persist", bufs=1) as pers,
            tc.tile_pool(name="stream", bufs=1) as stream,
            tc.tile_pool(name="evict", bufs=1) as evict,
            tc.tile_pool(name="psum", bufs=1, space="PSUM") as psp,
            tc.tile_pool(name="dram", bufs=1, space="DRAM") as dp,
        ):
            ident = constp.tile([128, 128], bf16, name="ident")
            masks.make_identity(nc, ident[:])

            a1T = [[pers.tile([128, S], bf16, name=f"a1T_{c}_{f}")
                    for f in range(4)] for c in range(3)]
            a2T = [[pers.tile([128, S], bf16, name=f"a2T_{c}_{f}")
                    for f in range(4)] for c in range(3)]
            ones_row = pers.tile([128, S], bf16, name="ones_row")
            nc.gpsimd.memset(ones_row[:], 0.0)
            nc.gpsimd.memset(ones_row[0:1, :], 1.0)
            for tset in (a1T, a2T):
                for c in range(3):
                    nc.gpsimd.memset(tset[c][3][:], 0.0)

            # bounce buffers: [2 tensors][5 mtiles][128][400] per half
            h23b = [dp.tile([2, KB, 128, H], bf16, name=f"h23b{h}")
                    for h in range(2)]
            h23g = [dp.tile([NP, H], bf16, name=f"h23g{h}",
                            addr_space="Shared") for h in range(2)]
            t3b = [dp.tile([KB, 128, H], bf16, name=f"t3b{h}")
                   for h in range(2)]
            t3g = [dp.tile([NCORES * KB * 128, H], bf16, name=f"t3g{h}",
                           addr_space="Shared") for h in range(2)]
            g23b = [dp.tile([2, KB, 128, H], bf16, name=f"g23b{h}")
                    for h in range(2)]
            g23g = [dp.tile([NP, H], bf16, name=f"g23g{h}",
                            addr_space="Shared") for h in range(2)]
            v3b = [dp.tile([KB, 128, H], bf16, name=f"v3b{h}")
                   for h in range(2)]
            v3g = [dp.tile([NCORES * KB * 128, H], bf16, name=f"v3g{h}",
                           addr_space="Shared") for h in range(2)]

            ccw_in = dp.tile([1, 128], bf16, name="ccw_in")
            ccw_out = dp.tile([NCORES, 128], bf16, name="ccw_out",
                              addr_space="Shared")
            nc.gpsimd.collective_compute(
                "AllGather", mybir.AluOpType.bypass, replica_groups=RG,
                ins=[ccw_in.opt()], outs=[ccw_out.opt()])

            def ag(inb, outb):
                nc.gpsimd.collective_compute(
                    "AllGather", mybir.AluOpType.bypass, replica_groups=RG,
                    ins=[inb.opt()], outs=[outb.opt()])

            def mmtile():
                return psp.tile([128, 512], f32, name="mm", tag="mm", bufs=6)

            NPRE = 20      # tiles of the next pass's rhs prefetched early

            def res_load(eng, pool, tag, nbufs, gbufs, row_of, ks, res):
                """Emit rhs resident-tile loads for k in ks on engine eng."""
                for k in ks:
                    b, kt = k // KB, k % KB
                    hf, r = b // NCORES, b % NCORES
                    t = pool.tile([128, H], bf16, name=tag, tag=tag,
                                  bufs=nbufs)
                    eng.dma_start(
                        t[:], gbufs[hf][row_of(r, kt):row_of(r, kt) + 128, :])
                    res.append(t)

            def row_half(r, kt):      # [rank][tensor][5 mtiles] gathers
                return r * S + kt * 128

            def row_third(r, kt):     # [rank][5 mtiles] gathers (t3/v3)
                return r * KB * 128 + kt * 128

            # ============ adjacency pass =================================
            def adj_pass(adj_d, groups, gath, res, width, out_cb):
                two = width == 2 * H
                g0s = np.cumsum([0] + groups).tolist()
                for g, G in enumerate(groups):
                    pss = {}
                    for j in range(G):
                        pss[(j, 0)] = mmtile()
                        if two:
                            pss[(j, 1)] = mmtile()
                    for b in range(NB):
                        hf, r = b // NCORES, b % NCORES
                        slab = stream.tile([128, KB * G * 128], bf16,
                                           name="slab", tag="slab", bufs=3)
                        nc.sync.dma_start(slab[:, 0:KB * G * 128],
                                          adj_d[g][b])
                        for kt in range(KB):
                            k = b * KB + kt
                            if two:
                                rt = stream.tile([128, H], bf16, name="rt",
                                                 tag="rt", bufs=8)
                                nc.scalar.dma_start(rt[:], gath(hf, r, kt))
                            for j in range(G):
                                lhs = slab[:, (kt * G + j) * 128:
                                           (kt * G + j + 1) * 128]
                                nc.tensor.matmul(
                                    pss[(j, 0)][:, 0:H], lhs, res[k][:],
                                    start=(k == 0), stop=(k == KT - 1))
                                if two:
                                    nc.tensor.matmul(
                                        pss[(j, 1)][:, 0:H], lhs, rt[:],
                                        start=(k == 0), stop=(k == KT - 1))
                    for j in range(G):
                        mt = g0s[g] + j
                        ot = evict.tile([128, width], bf16, name="po",
                                        tag="po", bufs=6)
                        nc.vector.tensor_copy(ot[:, 0:H], pss[(j, 0)][:, 0:H])
                        if two:
                            nc.vector.tensor_copy(ot[:, H:2 * H],
                                                  pss[(j, 1)][:, 0:H])
                        out_cb(mt, ot)

            def transpose_into(chunk_tiles, mt, src):
                for (h0, hw) in HC:
                    tp = psp.tile([128, 128], bf16, name="tp", tag="tp",
                                  bufs=2)
                    nc.tensor.transpose(tp[0:hw, :], src[:, h0:h0 + hw],
                                        ident[:])
                    nc.vector.tensor_copy(
                        chunk_tiles[h0 // 128][0:hw,
                                               mt * 128:(mt + 1) * 128],
                        tp[0:hw, :])

            # ================= phase 0 + pass A =========================
            with tc.tile_pool(name="resA", bufs=1) as resA:
                h2res = []
                with tc.tile_pool(name="ph0", bufs=1) as ph0:
                    xT = [ph0.tile([128, S], bf16, name=f"xT{k}")
                          for k in range(4)]
                    for k in range(4):
                        nc.sync.dma_start(xT[k][:],
                                          xT_d[k * 128:(k + 1) * 128, :])
                    wu = [ph0.tile([128, 4, H], bf16, name=f"wu{i}")
                          for i in range(3)]
                    for i in range(3):
                        nc.sync.dma_start(
                            wu[i][:],
                            wu_d[i].rearrange("(t p) h -> p t h", p=128))

                    def h23_half(hf):
                        for mt in range(hf * KB, hf * KB + KB):
                            for i in (1, 2):
                                ps = mmtile()
                                for k in range(4):
                                    nc.tensor.matmul(
                                        ps[:, 0:H],
                                        xT[k][:, mt * 128:(mt + 1) * 128],
                                        wu[i][:, k, :],
                                        start=(k == 0), stop=(k == 3))
                                hs = evict.tile([128, H], bf16, name="hs",
                                                tag="hs", bufs=5)
                                nc.scalar.activation(hs[:], ps[:, 0:H], RELU)
                                nc.scalar.dma_start(
                                    h23b[hf][i - 1, mt % KB], hs[:])
                        ag(h23b[hf], h23g[hf])
                        # resident h2 loads land on gpsimd right behind this
                        # gather's completion wait
                        res_load(nc.gpsimd, resA, "h2res", KT, h23g,
                                 row_half,
                                 range(hf * KT // 2, (hf + 1) * KT // 2),
                                 h2res)

                    h23_half(0)
                    h23_half(1)
                    # h1^T while the gathers run
                    for (h0, hw) in HC:
                        for (n0, nw) in NCH:
                            ps = mmtile()
                            for k in range(4):
                                nc.tensor.matmul(
                                    ps[0:hw, 0:nw],
                                    wu[0][:, k, h0:h0 + hw],
                                    xT[k][:, n0:n0 + nw],
                                    start=(k == 0), stop=(k == 3))
                            nc.scalar.activation(
                                a1T[0][h0 // 128][0:hw, n0:n0 + nw],
                                ps[0:hw, 0:nw], RELU)

                t3pre = []

                def passA_out(mt, ot):
                    transpose_into(a1T[1], mt, ot[:, 0:H])       # t2^T
                    nc.scalar.dma_start(t3b[mt // KB][mt % KB],
                                        ot[:, H:2 * H])
                    if mt == KB - 1:
                        ag(t3b[0], t3g[0])
                        res_load(nc.gpsimd, stream, "pre", NPRE, t3g,
                                 row_third, range(NPRE), t3pre)
                    elif mt == 2 * KB - 1:
                        ag(t3b[1], t3g[1])

                adj_pass(adjA_d, GA,
                         lambda hf, r, kt:
                         h23g[hf][r * S + KB * 128 + kt * 128:
                                  r * S + KB * 128 + kt * 128 + 128, :],
                         h2res, 2 * H, passA_out)

            # ================= pass B: u3 = adj @ t3 ====================
            with tc.tile_pool(name="resB", bufs=1) as resB:
                t3res = list(t3pre)
                # rest of half 0 on sync (no gather wait: t3g[0] is done)
                res_load(nc.sync, resB, "t3res", KT - NPRE, t3g, row_third,
                         range(NPRE, KT // 2), t3res)
                # half 1 on gpsimd, behind the AG_tb completion wait
                res_load(nc.gpsimd, resB, "t3res", KT - NPRE, t3g, row_third,
                         range(KT // 2, KT), t3res)

                def passB_out(mt, ot):
                    transpose_into(a1T[2], mt, ot[:, 0:H])       # u3^T

                adj_pass(adjB_d, GB, None, t3res, H, passB_out)

            # ================= layer-2 feature matmuls ==================
            g2pre = []
            with tc.tile_pool(name="wbp", bufs=1) as wbp:
                wb = [wbp.tile([128, CKT, H], bf16, name=f"wb{i}")
                      for i in range(3)]
                for i in range(3):
                    nc.scalar.dma_start(
                        wb[i][:], wb_d[i].rearrange("(t p) h -> p t h", p=128))
                a1k = [a1T[ck // 4][ck % 4] for ck in range(CKT)]

                def g23_half(hf):
                    for mt in range(hf * KB, hf * KB + KB):
                        for i in (1, 2):
                            ps = mmtile()
                            for ck in range(CKT):
                                nc.tensor.matmul(
                                    ps[:, 0:H],
                                    a1k[ck][:, mt * 128:(mt + 1) * 128],
                                    wb[i][:, ck, :],
                                    start=(ck == 0), stop=(ck == CKT - 1))
                            gs = evict.tile([128, H], bf16, name="hs",
                                            tag="hs", bufs=5)
                            nc.vector.tensor_copy(gs[:], ps[:, 0:H])
                            nc.scalar.dma_start(
                                g23b[hf][i - 1, mt % KB], gs[:])
                    ag(g23b[hf], g23g[hf])
                    if hf == 0:
                        res_load(nc.gpsimd, stream, "pre", NPRE, g23g,
                                 row_half, range(NPRE), g2pre)

                g23_half(0)
                g23_half(1)
                # g1^T while gathers run
                for (h0, hw) in HC:
                    for (n0, nw) in NCH:
                        ps = mmtile()
                        for ck in range(CKT):
                            nc.tensor.matmul(
                                ps[0:hw, 0:nw],
                                wb[0][:, ck, h0:h0 + hw],
                                a1k[ck][:, n0:n0 + nw],
                                start=(ck == 0), stop=(ck == CKT - 1))
                        nc.vector.tensor_copy(
                            a2T[0][h0 // 128][0:hw, n0:n0 + nw],
                            ps[0:hw, 0:nw])

            # ================= pass C: v23 = adj @ [g2 | g3] ============
            with tc.tile_pool(name="resC", bufs=1) as resC:
                g2res = list(g2pre)
                res_load(nc.sync, resC, "g2res", KT - NPRE, g23g, row_half,
                         range(NPRE, KT // 2), g2res)
                res_load(nc.gpsimd, resC, "g2res", KT - NPRE, g23g, row_half,
                         range(KT // 2, KT), g2res)

                v3pre = []

                def passC_out(mt, ot):
                    transpose_into(a2T[1], mt, ot[:, 0:H])       # v2^T
                    nc.scalar.dma_start(v3b[mt // KB][mt % KB],
                                        ot[:, H:2 * H])
                    if mt == KB - 1:
                        ag(v3b[0], v3g[0])
                        res_load(nc.gpsimd, stream, "pre", NPRE, v3g,
                                 row_third, range(NPRE), v3pre)
                    elif mt == 2 * KB - 1:
                        ag(v3b[1], v3g[1])

                adj_pass(adjA_d, GA,
                         lambda hf, r, kt:
                         g23g[hf][r * S + KB * 128 + kt * 128:
                                  r * S + KB * 128 + kt * 128 + 128, :],
                         g2res, 2 * H, passC_out)

            # ================= pass D: w3 = adj @ v3 ====================
            with tc.tile_pool(name="resD", bufs=1) as resD:
                v3res = list(v3pre)
                res_load(nc.sync, resD, "v3res", KT - NPRE, v3g, row_third,
                         range(NPRE, KT // 2), v3res)
                res_load(nc.gpsimd, resD, "v3res", KT - NPRE, v3g, row_third,
                         range(KT // 2, KT), v3res)

                def passD_out(mt, ot):
                    transpose_into(a2T[2], mt, ot[:, 0:H])       # w3^T

                adj_pass(adjB_d, GB, None, v3res, H, passD_out)

            # ================= final: relu(a2 @ W_fc + b_fc) ============
            with tc.tile_pool(name="wfcp", bufs=1) as wfcp:
                wfc = wfcp.tile([128, CKT + 1, C], bf16, name="wfc")
                nc.sync.dma_start(
                    wfc[:], wfc_d.rearrange("(t p) h -> p t h", p=128))
                a2k = [a2T[ck // 4][ck % 4] for ck in range(CKT)] + [ones_row]
                for mt in range(MT):
                    pss = [mmtile() for _ in range(3)]
                    for ck in range(CKT + 1):
                        for ci in range(3):
                            nc.tensor.matmul(
                                pss[ci][:, 0:H],
                                a2k[ck][:, mt * 128:(mt + 1) * 128],
                                wfc[:, ck, ci * H:(ci + 1) * H],
                                start=(ck == 0), stop=(ck == CKT))
                    ofin = wfcp.tile([128, C], f32, name="ofin",
                                     tag="ofin", bufs=3)
                    for ci in range(3):
                        nc.scalar.activation(
                            ofin[:, ci * H:(ci + 1) * H], pss[ci][:, 0:H],
                            RELU)
                    nc.gpsimd.dma_start(out_d[mt * 128:(mt + 1) * 128, :],
                                        ofin[:])

    nc.compile()
    return nc


def _prep_host(features, adj, Wu1, Wu2, Wu3, Wb1, Wb2, Wb3, W_fc, b_fc):
    """Pad / permute / tile / cast inputs, build per-core in_maps."""
    # contraction (source-node) permutation: position (h, r, j) -> node
    perm = (np.arange(NCORES)[None, :, None] * S
            + np.arange(2)[:, None, None] * (KB * 128)
            + np.arange(KB * 128)[None, None, :]).reshape(-1)

    adjT = np.zeros((NP, NP), dtype=BF16)
    adjT[:N, :N] = np.ascontiguousarray(adj.T).astype(BF16)
    adjT = adjT[perm, :]

    xT = np.zeros((FP, NP), dtype=BF16)
    xT[:F, :N] = features.T.astype(BF16)

    wu = np.zeros((3, FP, H), dtype=BF16)
    for i, W in enumerate((Wu1, Wu2, Wu3)):
        wu[i, :F, :] = W.astype(BF16)

    wb = np.zeros((3, 3 * CH, H), dtype=BF16)
    for i, W in enumerate((Wb1, Wb2, Wb3)):
        for c in range(3):
            wb[i, c * CH:c * CH + H, :] = W[c * H:(c + 1) * H, :].astype(BF16)

    wfc = np.zeros((3 * CH + 128, C), dtype=BF16)
    for c in range(3):
        wfc[c * CH:c * CH + H, :] = W_fc[c * H:(c + 1) * H, :].astype(BF16)
    wfc[3 * CH, :] = b_fc.astype(BF16)

    def tile_adj(shard, groups):
        """shard [NP, S] -> per-group [NB, 128, KB*G*128] contiguous."""
        out = []
        m0 = 0
        for G in groups:
            blk = shard[:, m0 * 128:(m0 + G) * 128]      # [NP, G*128]
            blk = blk.reshape(NB, KB, 128, G * 128).transpose(0, 2, 1, 3)
            out.append(np.ascontiguousarray(
                blk.reshape(NB, 128, KB * G * 128)))
            m0 += G
        return out

    in_maps = []
    for c in range(NCORES):
        shard = adjT[:, c * S:(c + 1) * S]
        m = {"xT": np.ascontiguousarray(xT[:, c * S:(c + 1) * S]),
             "wu": wu, "wb": wb, "wfc": wfc}
        for g, arr in enumerate(tile_adj(shard, GA)):
            m[f"adjA{g}"] = arr
        for g, arr in enumerate(tile_adj(shard, GB)):
            m[f"adjB{g}"] = arr
        in_maps.append(m)
    return in_maps


def get_compiled():
    global _compiled
    if _compiled is None:
        _compiled = _build()
    return _compiled


def kernel(features, adj, Wu1, Wu2, Wu3, Wb1, Wb2, Wb3, W_fc, b_fc,
           trace=False, **run_kwargs):
    from concourse.bass_utils import run_bass_kernel_spmd

    features = np.asarray(features, dtype=np.float32)
    adj = np.asarray(adj, dtype=np.float32)
    in_maps = _prep_host(features, adj,
                         np.asarray(Wu1), np.asarray(Wu2), np.asarray(Wu3),
                         np.asarray(Wb1), np.asarray(Wb2), np.asarray(Wb3),
                         np.asarray(W_fc), np.asarray(b_fc))
    nc = get_compiled()
    res = run_bass_kernel_spmd(nc, in_maps, core_ids=list(range(NCORES)),
                               trace=trace, **run_kwargs)
    out = np.concatenate([res.results[c]["out"] for c in range(NCORES)],
                         axis=0)[:N]
    kernel.last_results = res
    return np.asarray(out, dtype=np.float32)


# revision 12
# speedup vs baseline: 1.0232x; 1.0232x over previous
"""MixHopNetwork Trainium2 kernel: 8-core SPMD Bass/Tile implementation.

Reference computation (N=10000 nodes, F=500, H=400, C=1200):
  h_i = relu(X @ Wu_i); a1 = [h1, adj@h2, adj@adj@h3]            (N, 1200)
  g_i = a1 @ Wb_i;      a2 = [g1, adj@g2, adj@adj@g3]            (N, 1200)
  out = relu(a2 @ W_fc + b_fc)                                   (N, 1200)

Distribution: nodes row-sharded over 8 cores (1280 rows each, padded
10000->10240).  adj is pre-transposed on the host so each core holds the
column-shard adjT[:, mine] with the contraction dim partition-major.
The six adj@h products collapse into 4 adjacency passes (A: adj@[h2|h3],
B: adj@t3, C: adj@[g2|g3], D: adj@v3) with AllGathers of each core's
activation shard between dependent passes.  All matmuls run in bf16 with
fp32 PSUM accumulation.

Overlap structure: the contraction (source-node) order is permuted to
[half][rank][640] so every boundary AllGather splits into two half-shard
gathers -- the first fires mid-pass and overlaps the producing pass's
second half; the consuming pass walks half-0 k-blocks first so the
second gather overlaps its first half.  adjT is stored pre-tiled per
(m-group, k-block) so every stationary-slab DMA is one contiguous read.
rhs operands that are reused across m-groups (h2/g2 halves, t3/v3) stay
SBUF-resident; only the h3/g3 halves are re-streamed per m-group.

Feature-contraction matmuls need feature-major activations; adjacency
passes produce node-major ones.  a1/a2 are assembled feature-major from
directly-transposed matmuls (h1^T, g1^T via out = W^T @ X^T) plus PE
transposes of t2/u3/v2/w3.  Each 400-row concat chunk is padded to 512
rows to keep tiles 128-aligned; matching weight rows are zero-padded on
the host, which also folds b_fc into W_fc as contraction row 1536
against a constant ones-row appended to a2T.
"""
import sys

for _p in ("/opt/trn_rl_repo", "/root/.axon_site/_ro/trn_rl_repo"):
    if _p not in sys.path:
        sys.path.insert(0, _p)

import numpy as np
import ml_dtypes

NCORES = 8
N = 10000            # nodes
NP = 10240           # padded nodes
S = NP // NCORES     # 1280 rows per core
MT = S // 128        # 10 node tiles per core
KT = NP // 128       # 80 contraction tiles
KB = 5               # k-tiles per k-block (640 rows)
NB = KT // KB        # 16 k-blocks = [2 halves][8 ranks]
F, FP = 500, 512
H = 400
C = 1200
CH = 512             # padded chunk height in a1T/a2T
CKT = 3 * CH // 128  # 12 k-tiles over packed feature dim

GA = [3, 2, 3, 2]    # m-tile groups for 800-wide passes (A, C)
GB = [5, 5]          # m-tile groups for 400-wide passes (B, D)

BF16 = ml_dtypes.bfloat16

_compiled = None


def _build():
    from concourse import bass, bacc, tile, mybir
    from concourse import masks

    f32 = mybir.dt.float32
    bf16 = mybir.dt.bfloat16
    RELU = mybir.ActivationFunctionType.Relu

    nc = bacc.Bacc("TRN2", target_bir_lowering=False, debug=False,
                   num_devices=NCORES)

    adjA_d = [nc.dram_tensor(f"adjA{g}", [NB, 128, KB * G * 128], bf16,
                             kind="ExternalInput") for g, G in enumerate(GA)]
    adjB_d = [nc.dram_tensor(f"adjB{g}", [NB, 128, KB * G * 128], bf16,
                             kind="ExternalInput") for g, G in enumerate(GB)]
    xT_d = nc.dram_tensor("xT", [FP, S], bf16, kind="ExternalInput")
    wu_d = nc.dram_tensor("wu", [3, FP, H], bf16, kind="ExternalInput")
    wb_d = nc.dram_tensor("wb", [3, 3 * CH, H], bf16, kind="ExternalInput")
    wfc_d = nc.dram_tensor("wfc", [3 * CH + 128, C], bf16,
                           kind="ExternalInput")
    out_d = nc.dram_tensor("out", [S, C], f32, kind="ExternalOutput")

    RG = [list(range(NCORES))]
    HC = [(0, 128), (128, 128), (256, 128), (384, 16)]   # H chunks
    NCH = [(0, 512), (512, 512), (1024, 256)]            # node chunks

    with tile.TileContext(nc) as tc:
        with (
            tc.tile_pool(name="const", bufs=1) as constp,
            tc.tile_pool(name="persist", bufs=1) as pers,
            tc.tile_pool(name="stream", bufs=1) as stream,
            tc.tile_pool(name="evict", bufs=1) as evict,
            tc.tile_pool(name="psum", bufs=1, space="PSUM") as psp,
            tc.tile_pool(name="dram", bufs=1, space="DRAM") as dp,
        ):
            ident = constp.tile([128, 128], bf16, name="ident")
            masks.make_identity(nc, ident[:])

            a1T = [[pers.tile([128, S], bf16, name=f"a1T_{c}_{f}")
                    for f in range(4)] for c in range(3)]
            a2T = [[pers.tile([128, S], bf16, name=f"a2T_{c}_{f}")
                    for f in range(4)] for c in range(3)]
            ones_row = pers.tile([128, S], bf16, name="ones_row")
            nc.gpsimd.memset(ones_row[:], 0.0)
            nc.gpsimd.memset(ones_row[0:1, :], 1.0)
            for tset in (a1T, a2T):
                for c in range(3):
                    nc.gpsimd.memset(tset[c][3][:], 0.0)

            # bounce buffers: [2 tensors][5 mtiles][128][400] per half
            h23b = [dp.tile([2, KB, 128, H], bf16, name=f"h23b{h}")
                    for h in range(2)]
            h23g = [dp.tile([NP, H], bf16, name=f"h23g{h}",
                            addr_space="Shared") for h in range(2)]
            t3b = [dp.tile([KB, 128, H], bf16, name=f"t3b{h}")
                   for h in range(2)]
            t3g = [dp.tile([NCORES * KB * 128, H], bf16, name=f"t3g{h}",
                           addr_space="Shared") for h in range(2)]
            g23b = [dp.tile([2, KB, 128, H], bf16, name=f"g23b{h}")
                    for h in range(2)]
            g23g = [dp.tile([NP, H], bf16, name=f"g23g{h}",
                            addr_space="Shared") for h in range(2)]
            v3b = [dp.tile([KB, 128, H], bf16, name=f"v3b{h}")
                   for h in range(2)]
            v3g = [dp.tile([NCORES * KB * 128, H], bf16, name=f"v3g{h}",
                           addr_space="Shared") for h in range(2)]

            ccw_in = dp.tile([1, 128], bf16, name="ccw_in")
            ccw_out = dp.tile([NCORES, 128], bf16, name="ccw_out",
                              addr_space="Shared")
            nc.gpsimd.collective_compute(
                "AllGather", mybir.AluOpType.bypass, replica_groups=RG,
                ins=[ccw_in.opt()], outs=[ccw_out.opt()])

            def ag(inb, outb):
                nc.gpsimd.collective_compute(
                    "AllGather", mybir.AluOpType.bypass, replica_groups=RG,
                    ins=[inb.opt()], outs=[outb.opt()])

            def mmtile():
                return psp.tile([128, 512], f32, name="mm", tag="mm", bufs=6)

            NPRE = 15      # tiles of the next pass's rhs prefetched early

            def res_load(eng, pool, tag, nbufs, gbufs, row_of, ks, res):
                """Emit rhs resident-tile loads for k in ks on engine eng."""
                for k in ks:
                    b, kt = k // KB, k % KB
                    hf, r = b // NCORES, b % NCORES
                    t = pool.tile([128, H], bf16, name=tag, tag=tag,
                                  bufs=nbufs)
                    eng.dma_start(
                        t[:], gbufs[hf][row_of(r, kt):row_of(r, kt) + 128, :])
                    res.append(t)

            def row_half(r, kt):      # [rank][tensor][5 mtiles] gathers
                return r * S + kt * 128

            def row_third(r, kt):     # [rank][5 mtiles] gathers (t3/v3)
                return r * KB * 128 + kt * 128

            # ============ adjacency pass =================================
            def adj_pass(adj_d, groups, gath, res, width, out_cb):
                two = width == 2 * H
                g0s = np.cumsum([0] + groups).tolist()
                for g, G in enumerate(groups):
                    pss = {}
                    for j in range(G):
                        pss[(j, 0)] = mmtile()
                        if two:
                            pss[(j, 1)] = mmtile()
                    for b in range(NB):
                        hf, r = b // NCORES, b % NCORES
                        slab = stream.tile([128, KB * G * 128], bf16,
                                           name="slab", tag="slab", bufs=4)
                        nc.sync.dma_start(slab[:, 0:KB * G * 128],
                                          adj_d[g][b])
                        for kt in range(KB):
                            k = b * KB + kt
                            if two:
                                rt = stream.tile([128, H], bf16, name="rt",
                                                 tag="rt", bufs=8)
                                nc.scalar.dma_start(rt[:], gath(hf, r, kt))
                            for j in range(G):
                                lhs = slab[:, (kt * G + j) * 128:
                                           (kt * G + j + 1) * 128]
                                nc.tensor.matmul(
                                    pss[(j, 0)][:, 0:H], lhs, res[k][:],
                                    start=(k == 0), stop=(k == KT - 1))
                                if two:
                                    nc.tensor.matmul(
                                        pss[(j, 1)][:, 0:H], lhs, rt[:],
                                        start=(k == 0), stop=(k == KT - 1))
                    for j in range(G):
                        mt = g0s[g] + j
                        ot = evict.tile([128, width], bf16, name="po",
                                        tag="po", bufs=6)
                        nc.vector.tensor_copy(ot[:, 0:H], pss[(j, 0)][:, 0:H])
                        if two:
                            nc.vector.tensor_copy(ot[:, H:2 * H],
                                                  pss[(j, 1)][:, 0:H])
                        out_cb(mt, ot)

            def transpose_into(chunk_tiles, mt, src):
                for (h0, hw) in HC:
                    tp = psp.tile([128, 128], bf16, name="tp", tag="tp",
                                  bufs=2)
                    nc.tensor.transpose(tp[0:hw, :], src[:, h0:h0 + hw],
                                        ident[:])
                    nc.vector.tensor_copy(
                        chunk_tiles[h0 // 128][0:hw,
                                               mt * 128:(mt + 1) * 128],
                        tp[0:hw, :])

            # ================= phase 0 + pass A =========================
            with tc.tile_pool(name="resA", bufs=1) as resA:
                h2res = []
                with tc.tile_pool(name="ph0", bufs=1) as ph0:
                    xT = [ph0.tile([128, S], bf16, name=f"xT{k}")
                          for k in range(4)]
                    for k in range(4):
                        nc.sync.dma_start(xT[k][:],
                                          xT_d[k * 128:(k + 1) * 128, :])
                    wu = [ph0.tile([128, 4, H], bf16, name=f"wu{i}")
                          for i in range(3)]
                    for i in range(3):
                        nc.sync.dma_start(
                            wu[i][:],
                            wu_d[i].rearrange("(t p) h -> p t h", p=128))

                    def h23_half(hf):
                        for mt in range(hf * KB, hf * KB + KB):
                            for i in (1, 2):
                                ps = mmtile()
                                for k in range(4):
                                    nc.tensor.matmul(
                                        ps[:, 0:H],
                                        xT[k][:, mt * 128:(mt + 1) * 128],
                                        wu[i][:, k, :],
                                        start=(k == 0), stop=(k == 3))
                                hs = evict.tile([128, H], bf16, name="hs",
                                                tag="hs", bufs=5)
                                nc.scalar.activation(hs[:], ps[:, 0:H], RELU)
                                nc.scalar.dma_start(
                                    h23b[hf][i - 1, mt % KB], hs[:])
                        ag(h23b[hf], h23g[hf])
                        # resident h2 loads land on gpsimd right behind this
                        # gather's completion wait
                        res_load(nc.gpsimd, resA, "h2res", KT, h23g,
                                 row_half,
                                 range(hf * KT // 2, (hf + 1) * KT // 2),
                                 h2res)

                    h23_half(0)
                    h23_half(1)
                    # h1^T while the gathers run
                    for (h0, hw) in HC:
                        for (n0, nw) in NCH:
                            ps = mmtile()
                            for k in range(4):
                                nc.tensor.matmul(
                                    ps[0:hw, 0:nw],
                                    wu[0][:, k, h0:h0 + hw],
                                    xT[k][:, n0:n0 + nw],
                                    start=(k == 0), stop=(k == 3))
                            nc.scalar.activation(
                                a1T[0][h0 // 128][0:hw, n0:n0 + nw],
                                ps[0:hw, 0:nw], RELU)

                t3pre = []

                def passA_out(mt, ot):
                    transpose_into(a1T[1], mt, ot[:, 0:H])       # t2^T
                    nc.scalar.dma_start(t3b[mt // KB][mt % KB],
                                        ot[:, H:2 * H])
                    if mt == KB - 1:
                        ag(t3b[0], t3g[0])
                        res_load(nc.gpsimd, stream, "pre", NPRE, t3g,
                                 row_third, range(NPRE), t3pre)
                    elif mt == 2 * KB - 1:
                        ag(t3b[1], t3g[1])

                adj_pass(adjA_d, GA,
                         lambda hf, r, kt:
                         h23g[hf][r * S + KB * 128 + kt * 128:
                                  r * S + KB * 128 + kt * 128 + 128, :],
                         h2res, 2 * H, passA_out)

            # ================= pass B: u3 = adj @ t3 ====================
            with tc.tile_pool(name="resB", bufs=1) as resB:
                t3res = list(t3pre)
                # rest of half 0 on sync (no gather wait: t3g[0] is done)
                res_load(nc.sync, resB, "t3res", KT - NPRE, t3g, row_third,
                         range(NPRE, KT // 2), t3res)
                # half 1 on gpsimd, behind the AG_tb completion wait
                res_load(nc.gpsimd, resB, "t3res", KT - NPRE, t3g, row_third,
                         range(KT // 2, KT), t3res)

                def passB_out(mt, ot):
                    transpose_into(a1T[2], mt, ot[:, 0:H])       # u3^T

                adj_pass(adjB_d, GB, None, t3res, H, passB_out)

            # ================= layer-2 feature matmuls ==================
            g2pre = []
            with tc.tile_pool(name="wbp", bufs=1) as wbp:
                wb = [wbp.tile([128, CKT, H], bf16, name=f"wb{i}")
                      for i in range(3)]
                for i in range(3):
                    nc.scalar.dma_start(
                        wb[i][:], wb_d[i].rearrange("(t p) h -> p t h", p=128))
                a1k = [a1T[ck // 4][ck % 4] for ck in range(CKT)]

                def g23_half(hf):
                    for mt in range(hf * KB, hf * KB + KB):
                        for i in (1, 2):
                            ps = mmtile()
                            for ck in range(CKT):
                                nc.tensor.matmul(
                                    ps[:, 0:H],
                                    a1k[ck][:, mt * 128:(mt + 1) * 128],
                                    wb[i][:, ck, :],
                                    start=(ck == 0), stop=(ck == CKT - 1))
                            gs = evict.tile([128, H], bf16, name="hs",
                                            tag="hs", bufs=5)
                            nc.vector.tensor_copy(gs[:], ps[:, 0:H])
                            nc.scalar.dma_start(
                                g23b[hf][i - 1, mt % KB], gs[:])
                    ag(g23b[hf], g23g[hf])
                    if hf == 0:
                        res_load(nc.gpsimd, stream, "pre", NPRE, g23g,
                                 row_half, range(NPRE), g2pre)

                g23_half(0)
                g23_half(1)
                # g1^T while gathers run
                for (h0, hw) in HC:
                    for (n0, nw) in NCH:
                        ps = mmtile()
                        for ck in range(CKT):
                            nc.tensor.matmul(
                                ps[0:hw, 0:nw],
                                wb[0][:, ck, h0:h0 + hw],
                                a1k[ck][:, n0:n0 + nw],
                                start=(ck == 0), stop=(ck == CKT - 1))
                        nc.vector.tensor_copy(
                            a2T[0][h0 // 128][0:hw, n0:n0 + nw],
                            ps[0:hw, 0:nw])

            # ================= pass C: v23 = adj @ [g2 | g3] ============
            with tc.tile_pool(name="resC", bufs=1) as resC:
                g2res = list(g2pre)
                res_load(nc.sync, resC, "g2res", KT - NPRE, g23g, row_half,
                         range(NPRE, KT // 2), g2res)
                res_load(nc.gpsimd, resC, "g2res", KT - NPRE, g23g, row_half,
                         range(KT // 2, KT), g2res)

                v3pre = []

                def passC_out(mt, ot):
                    transpose_into(a2T[1], mt, ot[:, 0:H])       # v2^T
                    nc.scalar.dma_start(v3b[mt // KB][mt % KB],
                                        ot[:, H:2 * H])
                    if mt == KB - 1:
                        ag(v3b[0], v3g[0])
                        res_load(nc.gpsimd, stream, "pre", NPRE, v3g,
                                 row_third, range(NPRE), v3pre)
                    elif mt == 2 * KB - 1:
                        ag(v3b[1], v3g[1])

                adj_pass(adjA_d, GA,
                         lambda hf, r, kt:
                         g23g[hf][r * S + KB * 128 + kt * 128:
                                  r * S + KB * 128 + kt * 128 + 128, :],
                         g2res, 2 * H, passC_out)

            # ================= pass D: w3 = adj @ v3 ====================
            with tc.tile_pool(name="resD", bufs=1) as resD:
                v3res = list(v3pre)
                res_load(nc.sync, resD, "v3res", KT - NPRE, v3g, row_third,
                         range(NPRE, KT // 2), v3res)
                res_load(nc.gpsimd, resD, "v3res", KT - NPRE, v3g, row_third,
                         range(KT // 2, KT), v3res)

                def passD_out(mt, ot):
                    transpose_into(a2T[2], mt, ot[:, 0:H])       # w3^T

                adj_pass(adjB_d, GB, None, v3res, H, passD_out)

            # ================= final: relu(a2 @ W_fc + b_fc) ============
            with tc.tile_pool(name="wfcp", bufs=1) as wfcp:
                wfc = wfcp.tile([128, CKT + 1, C], bf16, name="wfc")
                nc.sync.dma_start(
                    wfc[:], wfc_d.rearrange("(t p) h -> p t h", p=128))
                a2k = [a2T[ck // 4][ck % 4] for ck in range(CKT)] + [ones_row]
                for mt in range(MT):
                    pss = [mmtile() for _ in range(3)]
                    for ck in range(CKT + 1):
                        for ci in range(3):
                            nc.tensor.matmul(
                                pss[ci][:, 0:H],
                                a2k[ck][:, mt * 128:(mt + 1) * 128],
                                wfc[:, ck, ci * H:(ci + 1) * H],
                                start=(ck == 0), stop=(ck == CKT))
                    ofin = wfcp.tile([128, C], f32, name="ofin",
                                     tag="ofin", bufs=3)
                    for ci in range(3):
                        nc.scalar.activation(
                            ofin[:, ci * H:(ci + 1) * H], pss[ci][:, 0:H],
                            RELU)
                    nc.gpsimd.dma_start(out_d[mt * 128:(mt + 1) * 128, :],
                                        ofin[:])

    nc.compile()
    return nc


def _prep_host(features, adj, Wu1, Wu2, Wu3, Wb1, Wb2, Wb3, W_fc, b_fc):
    """Pad / permute / tile / cast inputs, build per-core in_maps."""
    # contraction (source-node) permutation: position (h, r, j) -> node
    perm = (np.arange(NCORES)[None, :, None] * S
            + np.arange(2)[:, None, None] * (KB * 128)
            + np.arange(KB * 128)[None, None, :]).reshape(-1)

    adjT = np.zeros((NP, NP), dtype=BF16)
    adjT[:N, :N] = np.ascontiguousarray(adj.T).astype(BF16)
    adjT = adjT[perm, :]

    xT = np.zeros((FP, NP), dtype=BF16)
    xT[:F, :N] = features.T.astype(BF16)

    wu = np.zeros((3, FP, H), dtype=BF16)
    for i, W in enumerate((Wu1, Wu2, Wu3)):
        wu[i, :F, :] = W.astype(BF16)

    wb = np.zeros((3, 3 * CH, H), dtype=BF16)
    for i, W in enumerate((Wb1, Wb2, Wb3)):
        for c in range(3):
            wb[i, c * CH:c * CH + H, :] = W[c * H:(c + 1) * H, :].astype(BF16)

    wfc = np.zeros((3 * CH + 128, C), dtype=BF16)
    for c in range(3):
        wfc[c * CH:c * CH + H, :] = W_fc[c * H:(c + 1) * H, :].astype(BF16)
    wfc[3 * CH, :] = b_fc.astype(BF16)

    def tile_adj(shard, groups):
        """shard [NP, S] -> per-group [NB, 128, KB*G*128] contiguous."""
        out = []
        m0 = 0
        for G in groups:
            blk = shard[:, m0 * 128:(m0 + G) * 128]      # [NP, G*128]
            blk = blk.reshape(NB, KB, 128, G * 128).transpose(0, 2, 1, 3)
            out.append(np.ascontiguousarray(
                blk.reshape(NB, 128, KB * G * 128)))
            m0 += G
        return out

    in_maps = []
    for c in range(NCORES):
        shard = adjT[:, c * S:(c + 1) * S]
        m = {"xT": np.ascontiguousarray(xT[:, c * S:(c + 1) * S]),
             "wu": wu, "wb": wb, "wfc": wfc}
        for g, arr in enumerate(tile_adj(shard, GA)):
            m[f"adjA{g}"] = arr
        for g, arr in enumerate(tile_adj(shard, GB)):
            m[f"adjB{g}"] = arr
        in_maps.append(m)
    return in_maps


def get_compiled():
    global _compiled
    if _compiled is None:
        _compiled = _build()
    return _compiled


def kernel(features, adj, Wu1, Wu2, Wu3, Wb1, Wb2, Wb3, W_fc, b_fc,
           trace=False, **run_kwargs):
    from concourse.bass_utils import run_bass_kernel_spmd

    features = np.asarray(features, dtype=np.float32)
    adj = np.asarray(adj, dtype=np.float32)
    in_maps = _prep_host(features, adj,
                         np.asarray(Wu1), np.asarray(Wu2), np.asarray(Wu3),
                         np.asarray(Wb1), np.asarray(Wb2), np.asarray(Wb3),
                         np.asarray(W_fc), np.asarray(b_fc))
    nc = get_compiled()
    res = run_bass_kernel_spmd(nc, in_maps, core_ids=list(range(NCORES)),
                               trace=trace, **run_kwargs)
    out = np.concatenate([res.results[c]["out"] for c in range(NCORES)],
                         axis=0)[:N]
    kernel.last_results = res
    return np.asarray(out, dtype=np.float32)
